# revision 20
# baseline (speedup 1.0000x reference)
"""Trainium2 Bass kernel for causal multi-head attention (v5: batch x heads).

Problem: B=2, C=2048, H=1024, 16 heads, head_dim=64, float32.
    qkv = x @ Wqkv.T + b ; causal softmax attention ; out = att @ Wo.T + b

Sharding over 8 NeuronCores: core c owns batch c//4 and heads
{4g..4g+3} where g = c%4 (batch data-parallel x head tensor-parallel).
Halving the per-core token count halves the x-in and y-out DMA traffic
vs pure head sharding (DMA activity steals SBUF bandwidth from the PE
XBUS and ~2x-slows matmuls while active).

Each core computes q/k/v for its 4 heads over its 2048 tokens, runs
causal attention for 8 (head-pair, q-quarter) slices, and applies a
PARTIAL output projection (contraction over its 256 head-dims). The
host unshard sums 4 partial [2048, 1024] outputs per batch and adds
Wo_b.

Per-core schedule: attention steps are PE-bound; QKV tiles and the
per-qt Wo matmuls drain from a filler queue between attention kb-steps.
Head pair P is laid out exactly like v4's 2 heads: S tile [128, 1024]
with pair-head-B's columns packed right after A's so exp is one
instruction per kb-step.
"""
import math
from collections import deque

import numpy as np

C, H, NH, HD = 2048, 1024, 16, 64
B = 2
NCORES = 8

_cache = {}


def _build():
    import concourse.bass as bass
    import concourse.bacc as bacc
    import concourse.tile as tile
    import concourse.mybir as mybir

    dt = mybir.dt
    f32 = dt.float32
    f32r = dt.float32r
    bf16 = dt.bfloat16
    AF = mybir.ActivationFunctionType

    nc = bacc.Bacc("TRN2", target_bir_lowering=False, debug=False,
                   enable_asserts=True, num_devices=NCORES)

    def din(name, shape, d=f32):
        return nc.dram_tensor(name, shape, d, kind="ExternalInput").ap()

    # per-core: 2048 tokens (one batch), 4 heads as 2 pairs P0/P1
    xTb = din("xTb", [512, 4096], bf16)       # [tt*128+p, hb*512+j], tt<4
    wqkb = din("wqkb", [128, 4096], bf16)     # [p, pr*2048 + hb*256+ob*128+o]
    wvb = din("wvb", [128, 2048], bf16)       # [p, pr*1024 + hb*128+o]
    qkb = din("qkb", [128, 4])                # bias cols: (q,k) x pair
    vbcol = din("vbcol", [128, 2])            # v bias col per pair
    masks128 = din("masks128", [128, 128], bf16)
    ident = din("ident", [128, 128], bf16)
    ones1x64 = din("ones1x64", [1, 64], f32r)
    wob = din("wob", [128, 2048], bf16)       # [p, pr*1024 + col] Wo^T rows
    y_out = nc.dram_tensor("y", [2048, 1024], bf16, kind="ExternalOutput").ap()

    with tile.TileContext(nc) as tc:
        ctx_lp = nc.allow_low_precision(
            reason="bf16 operands; all matmuls accumulate in f32 PSUM")
        ctx_lp.__enter__()
        with (
            tc.tile_pool(name="const", bufs=1) as const_pool,
            tc.tile_pool(name="persist", bufs=1) as persist,
            tc.tile_pool(name="qkvps", bufs=2, space="PSUM") as qkv_ps,
            tc.tile_pool(name="sps", bufs=2, space="PSUM") as s_ps,
            tc.tile_pool(name="avps", bufs=1, space="PSUM") as av_ps,
            tc.tile_pool(name="xt", bufs=4) as xt_pool,
            tc.tile_pool(name="psb", bufs=8) as p_pool,
            tc.tile_pool(name="attsb", bufs=8) as att_pool,
            tc.tile_pool(name="recsb", bufs=2) as rec_pool,
            tc.tile_pool(name="ysb", bufs=3) as y_pool,
        ):
            # -------- weights + first inputs, chunk-interleaved in FIRST-USE
            # order on one queue so the first QKV chain starts ~1us in and
            # never starves: item (pr0,q/k,h0) needs wqk[256*h0 : +512] and
            # xt0 chunk h0/2.
            qkb_sb = const_pool.tile([128, 4], f32, tag="qkb")
            nc.sync.dma_start(qkb_sb[:], qkb)
            # preload the exp table set during startup DMA wait
            dummy = rec_pool.tile([1, 2], f32, tag="dummy", name="dummy")
            nc.scalar.activation(dummy[:], qkb_sb[0:1, 0:2], AF.Exp)
            wqk_sb = const_pool.tile([128, 4096], bf16, tag="wqk")
            wv_sb = const_pool.tile([128, 2048], bf16, tag="wv")
            vb_sb = const_pool.tile([128, 2], f32, tag="vb")
            ident_sb = const_pool.tile([128, 128], bf16, tag="ident")
            xt0 = xt_pool.tile([128, 4096], bf16, tag="xt", name="xt")

            # Two concurrent startup streams while the PE is still idle:
            # sync carries weights (first-use order), gpsimd carries xt0.
            for c0, c1 in ((0, 512), (512, 1024), (1024, 1536), (1536, 2048),
                           (2048, 3072), (3072, 4096)):
                nc.sync.dma_start(wqk_sb[:, c0:c1], wqkb[:, c0:c1])
            for c4 in range(4):
                nc.gpsimd.dma_start(
                    xt0[:, 1024 * c4:1024 * c4 + 1024],
                    xTb[0:128, 1024 * c4:1024 * c4 + 1024])
            nc.sync.dma_start(wv_sb[:, 0:1024], wvb[:, 0:1024])
            nc.sync.dma_start(wv_sb[:, 1024:2048], wvb[:, 1024:2048])
            nc.sync.dma_start(vb_sb[:], vbcol)
            nc.sync.dma_start(ident_sb[:], ident)
            masks_sb = const_pool.tile([128, 128], bf16, tag="masks")
            nc.gpsimd.dma_start(masks_sb[:], masks128)
            ones_sb = const_pool.tile([1, 64], f32r, tag="ones")
            nc.gpsimd.dma_start(ones_sb[:], ones1x64)
            wob_sb = const_pool.tile([128, 2048], bf16, tag="wob")
            nc.gpsimd.dma_start(wob_sb[:], wob)

            # -------- persistent activations (per pair: [128 dims, 2048 tok])
            qT_sb = [persist.tile([128, 2048], bf16, tag=f"qT{p}",
                                  name=f"qT{p}") for p in (0, 1)]
            kT_sb = [persist.tile([128, 2048], bf16, tag=f"kT{p}",
                                  name=f"kT{p}") for p in (0, 1)]
            vT_sb = [persist.tile([128, 2048], bf16, tag=f"vT{p}",
                                  name=f"vT{p}") for p in (0, 1)]
            # v slots (width 256 per (pair, tb)): [vA|ones|pad|vB|ones|pad]
            # slot index = 16*pr + tb
            v_sb = persist.tile([128, 32 * 256], bf16, tag="v")
            ones_view = v_sb[:].rearrange("p (s h e) -> p s h e", s=32, h=2, e=128)
            nc.gpsimd.memset(ones_view[:, :, :, 64], 1.0)

            # ---------------- xt prefetch (single stream, just in time)
            xt_tiles = {0: xt0}

            def prefetch_xt(tt):
                xt = xt_pool.tile([128, 4096], bf16, tag="xt", name="xt")
                nc.sync.dma_start(xt[:], xTb[128 * tt:128 * tt + 128, :])
                xt_tiles[tt] = xt

            # ---------------- filler queue ----------------
            filler = deque()  # (pe_cost_ns, emit_fn, tag)
            emitted_tags = set()

            def drain(budget_ns):
                while budget_ns > 0 and filler:
                    cost, fn, tag = filler.popleft()
                    fn()
                    if tag is not None:
                        emitted_tags.add(tag)
                    budget_ns -= cost

            def drain_all():
                drain(float("inf"))

            def drain_until(tag):
                while tag not in emitted_tags and filler:
                    cost, fn, t = filler.popleft()
                    fn()
                    if t is not None:
                        emitted_tags.add(t)

            # ---------------- building blocks ----------------
            def enqueue_qkv(tt):
                """QKV for one 512-token tile: 6 chains (q,k,v x 2 pairs),
                each 8 accumulating matmuls of 512 cols."""
                xt = xt_tiles[tt]
                for pr in range(2):
                    for ob in range(3):  # q, k, v
                        ps_box = []

                        def make_item(pr, ob, h0, ps_box=None, xt=xt):
                            def emit():
                                if h0 == 0:
                                    ps_box.append(qkv_ps.tile(
                                        [128, 512], f32, tag="qkv",
                                        name="qkvp"))
                                ps = ps_box[0]
                                for hb in (h0, h0 + 1):
                                    if ob < 2:
                                        w = wqk_sb[:, 2048 * pr + 256 * hb
                                                   + 128 * ob:
                                                   2048 * pr + 256 * hb
                                                   + 128 * ob + 128]
                                    else:
                                        w = wv_sb[:, 1024 * pr + 128 * hb:
                                                  1024 * pr + 128 * hb + 128]
                                    nc.tensor.matmul(
                                        ps[:], w,
                                        xt[:, 512 * hb:512 * hb + 512],
                                        start=(hb == 0), stop=(hb == 7))
                                if h0 == 6:
                                    dst = (qT_sb, kT_sb, vT_sb)[ob][pr]
                                    if ob < 2:
                                        bias = qkb_sb[:, 2 * pr + ob:
                                                      2 * pr + ob + 1]
                                    else:
                                        bias = vb_sb[:, pr:pr + 1]
                                    nc.vector.tensor_scalar_add(
                                        dst[:, 512 * tt:512 * tt + 512],
                                        ps[:], bias)
                            return emit

                        for h0 in (0, 2, 4, 6):
                            filler.append(
                                (430, make_item(pr, ob, h0, ps_box),
                                 ("qkv", tt) if (pr == 1 and ob == 2
                                                 and h0 == 6) else None))

            def v_transpose(pr, tbs):
                """vT [d, t] -> v slots [t, d]: PE transpose + one strided
                DVE copy per 128-token block."""
                for tb in tbs:
                    slot = 16 * pr + tb
                    tcols = slice(128 * tb, 128 * tb + 128)
                    ps = qkv_ps.tile([128, 128], bf16, tag="qkv", name="vt")
                    nc.tensor.transpose(ps[:], vT_sb[pr][:, tcols], ident_sb[:])
                    dst = v_sb[:, 256 * slot:256 * slot + 256].rearrange(
                        "p (h x) -> p h x", h=2, x=128)[:, :, 0:64]
                    src = ps[:].rearrange("p (h x) -> p h x", h=2, x=64)
                    nc.vector.tensor_copy(dst, src)

            def enqueue_wo(att0, att1, qt):
                """Partial Wo for one q-quarter: contraction over BOTH pairs'
                att (256 dims): 2 accumulating matmuls per psum half."""
                for tb in range(4):
                    def make_item(tb, att0=att0, att1=att1, qt=qt):
                        def emit():
                            ps0 = qkv_ps.tile([128, 512], f32, tag="qkv",
                                              name="yps0")
                            nc.tensor.matmul(
                                ps0[:], att0[:, 128 * tb:128 * tb + 128],
                                wob_sb[:, 0:512], start=True, stop=False)
                            nc.tensor.matmul(
                                ps0[:], att1[:, 128 * tb:128 * tb + 128],
                                wob_sb[:, 1024:1536], start=False, stop=True)
                            ps1 = qkv_ps.tile([128, 512], f32, tag="qkv",
                                              name="yps1")
                            nc.tensor.matmul(
                                ps1[:], att0[:, 128 * tb:128 * tb + 128],
                                wob_sb[:, 512:1024], start=True, stop=False)
                            nc.tensor.matmul(
                                ps1[:], att1[:, 128 * tb:128 * tb + 128],
                                wob_sb[:, 1536:2048], start=False, stop=True)
                            ysb = y_pool.tile([128, 1024], bf16, tag="ysb",
                                              name="ysb")
                            nc.scalar.copy(ysb[:, 0:512], ps0[:])
                            nc.vector.tensor_copy(ysb[:, 512:1024], ps1[:])
                            row0 = 512 * qt + 128 * tb
                            eng = nc.gpsimd if tb % 2 == 0 else nc.sync
                            eng.dma_start(y_out[row0:row0 + 128, :], ysb[:])
                        return emit
                    filler.append((900, make_item(tb), None))

            def attention_slice(pr, qt):
                """Causal attention for head-pair pr, q rows [512qt, 512qt+512).
                Returns the normalized att tile [128 dims, 512 q]."""
                nkb = 4 * (qt + 1)
                avA = av_ps.tile([65, 512], f32, tag="avA", name="avA")
                avB = av_ps.tile([65, 512], f32, tag="avB", name="avB")
                qlo = 512 * qt
                pend = None  # AV runs one kb behind S^T/exp
                for kb in range(nkb):
                    klo = 128 * kb
                    roff = kb - 4 * qt
                    lo = 128 * roff if roff > 0 else 0
                    sAB = s_ps.tile([128, 1024], f32, tag="s", name="sAB")
                    nc.tensor.matmul(
                        sAB[:, lo:512], kT_sb[pr][0:64, klo:klo + 128],
                        qT_sb[pr][0:64, qlo + lo:qlo + 512])
                    nc.tensor.matmul(
                        sAB[:, 512:1024 - lo], kT_sb[pr][64:128, klo:klo + 128],
                        qT_sb[pr][64:128, qlo + lo:qlo + 512])
                    pAB = p_pool.tile([128, 1024], bf16, tag="p", name="pAB")
                    nc.scalar.activation(pAB[:, lo:1024 - lo],
                                         sAB[:, lo:1024 - lo],
                                         AF.Exp, scale=1.0 / math.sqrt(HD))
                    if roff >= 0:
                        c0 = 128 * roff
                        nc.gpsimd.tensor_mul(pAB[:, c0:c0 + 128],
                                             pAB[:, c0:c0 + 128], masks_sb[:])
                        nc.gpsimd.tensor_mul(pAB[:, 512:640],
                                             pAB[:, 512:640], masks_sb[:])
                    if pend is not None:
                        pkb, ppAB, plo = pend
                        pslot = 16 * pr + pkb
                        nc.tensor.matmul(
                            avA[:, plo:512],
                            v_sb[:, 256 * pslot:256 * pslot + 65],
                            ppAB[:, plo:512], start=(pkb == 0), stop=False)
                        nc.tensor.matmul(
                            avB[:, plo:512],
                            v_sb[:, 256 * pslot + 128:256 * pslot + 193],
                            ppAB[:, 512:1024 - plo], start=(pkb == 0),
                            stop=False)
                    pend = (kb, pAB, lo)
                    drain(900)
                pkb, ppAB, plo = pend
                pslot = 16 * pr + pkb
                nc.tensor.matmul(
                    avA[:, plo:512], v_sb[:, 256 * pslot:256 * pslot + 65],
                    ppAB[:, plo:512], start=(pkb == 0), stop=True)
                nc.tensor.matmul(
                    avB[:, plo:512], v_sb[:, 256 * pslot + 128:256 * pslot + 193],
                    ppAB[:, 512:1024 - plo], start=(pkb == 0), stop=True)

                # normalize: att_h = av_h[0:64] / sum_h  (sums in row 64).
                # rcp goes through qkv_ps (not s_ps) so the NEXT slice's S
                # matmuls don't wait on this slice's normalize chain.
                lrowA = rec_pool.tile([1, 512], f32r, tag="lrowA", name="lrowA")
                nc.scalar.copy(lrowA[:], avA[64:65, :])
                lrowB = rec_pool.tile([1, 512], f32r, tag="lrowB", name="lrowB")
                nc.scalar.copy(lrowB[:], avB[64:65, :])
                rcpA_ps = qkv_ps.tile([64, 512], f32, tag="qkv", name="rcpA_ps")
                nc.tensor.matmul(rcpA_ps[:], ones_sb[:], lrowA[:],
                                 start=True, stop=True)
                rcpB_ps = qkv_ps.tile([64, 512], f32, tag="qkv", name="rcpB_ps")
                nc.tensor.matmul(rcpB_ps[:], ones_sb[:], lrowB[:],
                                 start=True, stop=True)
                rcp_sb = rec_pool.tile([64, 1024], f32, tag="rcp", name="rcp_sb")
                nc.vector.reciprocal_approx_fast(rcp_sb[:, 0:512], rcpA_ps[:])
                nc.vector.reciprocal_approx_fast(rcp_sb[:, 512:1024], rcpB_ps[:])
                att_sb = att_pool.tile([128, 512], bf16, tag="att", name="att")
                nc.vector.tensor_mul(att_sb[0:64, :], avA[0:64, :],
                                     rcp_sb[:, 0:512])
                tmpB = rec_pool.tile([64, 512], bf16, tag="tmpB", name="tmpB")
                nc.vector.tensor_mul(tmpB[:], avB[0:64, :], rcp_sb[:, 512:1024])
                nc.vector.tensor_copy(att_sb[64:128, :], tmpB[:])
                return att_sb

            # ---------------- emission ----------------
            enqueue_qkv(0)
            drain_all()
            for qt in range(4):
                if qt < 3:
                    if qt + 1 not in xt_tiles:
                        prefetch_xt(qt + 1)
                    enqueue_qkv(qt + 1)
                drain_until(("qkv", qt))
                v_transpose(0, range(4 * qt, 4 * qt + 4))
                att0 = attention_slice(0, qt)
                v_transpose(1, range(4 * qt, 4 * qt + 4))
                att1 = attention_slice(1, qt)
                enqueue_wo(att0, att1, qt)
            drain_all()
        ctx_lp.__exit__(None, None, None)

    nc.compile()
    return nc


def host_prep(x, Wqkv_w, Wqkv_b, Wo_w, Wo_b):
    import ml_dtypes
    bf16 = ml_dtypes.bfloat16

    x = np.asarray(x, np.float32)
    Wqkv_w = np.asarray(Wqkv_w, np.float32)
    Wqkv_b = np.asarray(Wqkv_b, np.float32)
    Wo_w = np.asarray(Wo_w, np.float32)

    masks = np.ascontiguousarray(
        (np.arange(128)[:, None] <= np.arange(128)[None, :])
    ).astype(bf16)
    ones1x64 = np.ones((1, 64), dtype=np.float32)
    identm = np.ascontiguousarray(np.eye(128, dtype=np.float32)).astype(bf16)

    # xTb per batch: [512, 4096] with row tt*128+p, col hb*512+j
    xTbs = []
    for b in range(B):
        xf = x[b]                              # [2048, 1024]
        xTbs.append(np.ascontiguousarray(
            xf.reshape(4, 512, 8, 128).transpose(0, 3, 2, 1).reshape(512, 4096)
        ).astype(bf16))

    in_maps = []
    for c in range(NCORES):
        b, g = c // 4, c % 4
        heads = [4 * g + j for j in range(4)]   # two pairs: (h0,h1),(h2,h3)
        wqk_parts, qkb_cols, wv_parts, vb_cols, wob_parts = [], [], [], [], []
        for pr in range(2):
            hA, hB = heads[2 * pr], heads[2 * pr + 1]
            rows_qk = np.r_[64 * hA:64 * hA + 64, 64 * hB:64 * hB + 64,
                            1024 + 64 * hA:1024 + 64 * hA + 64,
                            1024 + 64 * hB:1024 + 64 * hB + 64]
            Wsub = Wqkv_w[rows_qk]              # [256, 1024]
            wqk_parts.append(
                Wsub.reshape(2, 128, 8, 128).transpose(3, 2, 0, 1)
                .reshape(128, 2048))
            qkb_cols.append(Wqkv_b[rows_qk].reshape(2, 128).T)  # [128, 2]
            rows_v = np.r_[2048 + 64 * hA:2048 + 64 * hA + 64,
                           2048 + 64 * hB:2048 + 64 * hB + 64]
            Vsub = Wqkv_w[rows_v]               # [128, 1024]
            wv_parts.append(
                Vsub.reshape(128, 8, 128).transpose(2, 1, 0).reshape(128, 1024))
            vb_cols.append(Wqkv_b[rows_v].reshape(128, 1))
            rows_o = np.r_[64 * hA:64 * hA + 64, 64 * hB:64 * hB + 64]
            wob_parts.append(Wo_w[:, rows_o].T)  # [128, 1024]
        wqkb = np.ascontiguousarray(
            np.concatenate(wqk_parts, axis=1)).astype(bf16)       # [128, 4096]
        qkbias = np.ascontiguousarray(np.concatenate(qkb_cols, axis=1))
        wvb = np.ascontiguousarray(
            np.concatenate(wv_parts, axis=1)).astype(bf16)        # [128, 2048]
        vbcol = np.ascontiguousarray(np.concatenate(vb_cols, axis=1))
        wob = np.ascontiguousarray(
            np.concatenate(wob_parts, axis=1)).astype(bf16)       # [128, 2048]
        in_maps.append(dict(
            xTb=xTbs[b], wqkb=wqkb, qkb=qkbias, wvb=wvb, vbcol=vbcol,
            masks128=masks, ones1x64=ones1x64, wob=wob, ident=identm))
    return in_maps


def _ensure_ntff_hook_module():
    """run_bass_kernel_spmd(trace=True) under axon imports
    antenv.axon_hooks; provide a ctypes-based fallback if absent."""
    import importlib
    import sys
    import types
    try:
        importlib.import_module("antenv.axon_hooks")
        return
    except ImportError:
        pass
    import contextlib
    import ctypes

    mod = types.ModuleType("antenv.axon_hooks")
    state = {"hook": None}

    def set_axon_ntff_profile_hook(h):
        state["hook"] = h

    def _make():
        try:
            lib = ctypes.CDLL("/opt/axon/libaxon_pjrt.so")
        except OSError:
            return None
        if not hasattr(lib, "axon_start_nrt_profile"):
            return None
        lib.axon_start_nrt_profile.argtypes = [
            ctypes.POINTER(ctypes.c_int64), ctypes.c_size_t]
        lib.axon_start_nrt_profile.restype = ctypes.c_int64
        lib.axon_stop_nrt_profile.argtypes = [ctypes.c_char_p]
        lib.axon_stop_nrt_profile.restype = ctypes.c_int64

        @contextlib.contextmanager
        def _hook(output_dir, device_ids):
            import jax
            jax.devices()
            if device_ids:
                ids = (ctypes.c_int64 * len(device_ids))(*device_ids)
                rc = lib.axon_start_nrt_profile(ids, len(device_ids))
            else:
                rc = lib.axon_start_nrt_profile(None, 0)
            if rc != 0:
                raise RuntimeError(f"axon_start_nrt_profile rc={rc}")
            try:
                yield
            finally:
                lib.axon_stop_nrt_profile(str(output_dir).encode())

        return _hook

    def get_axon_ntff_profile_hook():
        if state["hook"] is None:
            state["hook"] = _make()
        return state["hook"]

    mod.set_axon_ntff_profile_hook = set_axon_ntff_profile_hook
    mod.get_axon_ntff_profile_hook = get_axon_ntff_profile_hook
    try:
        import antenv
        sys.modules["antenv.axon_hooks"] = mod
        antenv.axon_hooks = mod
    except ImportError:
        pass


def kernel(x, Wqkv_w, Wqkv_b, Wo_w, Wo_b):
    from concourse import bass_utils

    _ensure_ntff_hook_module()

    if "nc" not in _cache:
        _cache["nc"] = _build()
    nc = _cache["nc"]

    in_maps = host_prep(x, Wqkv_w, Wqkv_b, Wo_w, Wo_b)
    res = bass_utils.run_bass_kernel_spmd(nc, in_maps, core_ids=list(range(NCORES)))
    _cache["last_results"] = res

    out = np.zeros((B, C, H), np.float32)
    for c in range(NCORES):
        b = c // 4
        out[b] += res.results[c]["y"].astype(np.float32)
    out += np.asarray(Wo_b, np.float32)[None, None, :]
    return out


# revision 22
# speedup vs baseline: 1.0470x; 1.0470x over previous
"""Trainium2 Bass kernel for causal multi-head attention (v5: batch x heads).

Problem: B=2, C=2048, H=1024, 16 heads, head_dim=64, float32.
    qkv = x @ Wqkv.T + b ; causal softmax attention ; out = att @ Wo.T + b

Sharding over 8 NeuronCores: core c owns batch c//4 and heads
{4g..4g+3} where g = c%4 (batch data-parallel x head tensor-parallel).
Halving the per-core token count halves the x-in and y-out DMA traffic
vs pure head sharding (DMA activity steals SBUF bandwidth from the PE
XBUS and ~2x-slows matmuls while active).

Each core computes q/k/v for its 4 heads over its 2048 tokens, runs
causal attention for 8 (head-pair, q-quarter) slices, and applies a
PARTIAL output projection (contraction over its 256 head-dims). The
host unshard sums 4 partial [2048, 1024] outputs per batch and adds
Wo_b.

Per-core schedule: attention steps are PE-bound; QKV tiles and the
per-qt Wo matmuls drain from a filler queue between attention kb-steps.
Head pair P is laid out exactly like v4's 2 heads: S tile [128, 1024]
with pair-head-B's columns packed right after A's so exp is one
instruction per kb-step.
"""
import math
from collections import deque

import numpy as np

C, H, NH, HD = 2048, 1024, 16, 64
B = 2
NCORES = 8

_cache = {}


def _build():
    import concourse.bass as bass
    import concourse.bacc as bacc
    import concourse.tile as tile
    import concourse.mybir as mybir

    dt = mybir.dt
    f32 = dt.float32
    f32r = dt.float32r
    bf16 = dt.bfloat16
    AF = mybir.ActivationFunctionType

    nc = bacc.Bacc("TRN2", target_bir_lowering=False, debug=False,
                   enable_asserts=True, num_devices=NCORES)

    def din(name, shape, d=f32):
        return nc.dram_tensor(name, shape, d, kind="ExternalInput").ap()

    # per-core: 2048 tokens (one batch), 4 heads as 2 pairs P0/P1
    xTb = din("xTb", [512, 4096], bf16)       # [tt*128+p, hb*512+j], tt<4
    wqkb = din("wqkb", [128, 4096], bf16)     # [p, pr*2048 + hb*256+ob*128+o]
    wvb = din("wvb", [128, 2048], bf16)       # [p, pr*1024 + hb*128+o]
    qkb = din("qkb", [128, 4])                # bias cols: (q,k) x pair
    vbcol = din("vbcol", [128, 2])            # v bias col per pair
    masks128 = din("masks128", [128, 128], bf16)
    ident = din("ident", [128, 128], bf16)
    ones1x64 = din("ones1x64", [1, 64], f32r)
    wob = din("wob", [128, 2048], bf16)       # [p, pr*1024 + col] Wo^T rows
    y_out = nc.dram_tensor("y", [2048, 1024], bf16, kind="ExternalOutput").ap()

    with tile.TileContext(nc) as tc:
        ctx_lp = nc.allow_low_precision(
            reason="bf16 operands; all matmuls accumulate in f32 PSUM")
        ctx_lp.__enter__()
        with (
            tc.tile_pool(name="const", bufs=1) as const_pool,
            tc.tile_pool(name="persist", bufs=1) as persist,
            tc.tile_pool(name="qkvps", bufs=2, space="PSUM") as qkv_ps,
            tc.tile_pool(name="sps", bufs=2, space="PSUM") as s_ps,
            tc.tile_pool(name="avps", bufs=1, space="PSUM") as av_ps,
            tc.tile_pool(name="xt", bufs=4) as xt_pool,
            tc.tile_pool(name="psb", bufs=8) as p_pool,
            tc.tile_pool(name="attsb", bufs=8) as att_pool,
            tc.tile_pool(name="recsb", bufs=2) as rec_pool,
            tc.tile_pool(name="ysb", bufs=3) as y_pool,
        ):
            # -------- weights + first inputs, chunk-interleaved in FIRST-USE
            # order on one queue so the first QKV chain starts ~1us in and
            # never starves: item (pr0,q/k,h0) needs wqk[256*h0 : +512] and
            # xt0 chunk h0/2.
            qkb_sb = const_pool.tile([128, 4], f32, tag="qkb")
            nc.sync.dma_start(qkb_sb[:], qkb)
            # preload the exp table set during startup DMA wait
            dummy = rec_pool.tile([1, 2], f32, tag="dummy", name="dummy")
            nc.scalar.activation(dummy[:], qkb_sb[0:1, 0:2], AF.Exp)
            wqk_sb = const_pool.tile([128, 4096], bf16, tag="wqk")
            wv_sb = const_pool.tile([128, 2048], bf16, tag="wv")
            vb_sb = const_pool.tile([128, 2], f32, tag="vb")
            ident_sb = const_pool.tile([128, 128], bf16, tag="ident")
            xt0 = xt_pool.tile([128, 4096], bf16, tag="xt", name="xt")

            # Startup DMAs chunk-interleaved in first-use order on one queue:
            # item (pr0, q/k, h0) needs wqk[256*h0 : +512] and xt0 chunk h0/2.
            def _c(dst, src, c0, c1):
                nc.sync.dma_start(dst[:, c0:c1], src[:, c0:c1])

            _c(wqk_sb, wqkb, 0, 512)
            nc.sync.dma_start(xt0[:, 0:1024], xTb[0:128, 0:1024])
            _c(wqk_sb, wqkb, 512, 1024)
            nc.sync.dma_start(xt0[:, 1024:2048], xTb[0:128, 1024:2048])
            _c(wqk_sb, wqkb, 1024, 1536)
            nc.sync.dma_start(xt0[:, 2048:3072], xTb[0:128, 2048:3072])
            _c(wqk_sb, wqkb, 1536, 2048)
            nc.sync.dma_start(xt0[:, 3072:4096], xTb[0:128, 3072:4096])
            _c(wqk_sb, wqkb, 2048, 3072)
            _c(wqk_sb, wqkb, 3072, 4096)
            _c(wv_sb, wvb, 0, 1024)
            _c(wv_sb, wvb, 1024, 2048)
            nc.sync.dma_start(vb_sb[:], vbcol)
            nc.sync.dma_start(ident_sb[:], ident)
            masks_sb = const_pool.tile([128, 128], bf16, tag="masks")
            nc.gpsimd.dma_start(masks_sb[:], masks128)
            ones_sb = const_pool.tile([1, 64], f32r, tag="ones")
            nc.gpsimd.dma_start(ones_sb[:], ones1x64)
            wob_sb = const_pool.tile([128, 2048], bf16, tag="wob")
            nc.gpsimd.dma_start(wob_sb[:], wob)

            # -------- persistent activations (per pair: [128 dims, 2048 tok])
            qT_sb = [persist.tile([128, 2048], bf16, tag=f"qT{p}",
                                  name=f"qT{p}") for p in (0, 1)]
            kT_sb = [persist.tile([128, 2048], bf16, tag=f"kT{p}",
                                  name=f"kT{p}") for p in (0, 1)]
            vT_sb = [persist.tile([128, 2048], bf16, tag=f"vT{p}",
                                  name=f"vT{p}") for p in (0, 1)]
            # v slots (width 256 per (pair, tb)): [vA|ones|pad|vB|ones|pad]
            # slot index = 16*pr + tb
            v_sb = persist.tile([128, 32 * 256], bf16, tag="v")
            ones_view = v_sb[:].rearrange("p (s h e) -> p s h e", s=32, h=2, e=128)
            nc.gpsimd.memset(ones_view[:, :, :, 64], 1.0)

            # ---------------- xt prefetch (single stream, just in time)
            xt_tiles = {0: xt0}

            def prefetch_xt(tt):
                xt = xt_pool.tile([128, 4096], bf16, tag="xt", name="xt")
                nc.sync.dma_start(xt[:], xTb[128 * tt:128 * tt + 128, :])
                xt_tiles[tt] = xt

            # ---------------- filler queue ----------------
            filler = deque()  # (pe_cost_ns, emit_fn, tag)
            emitted_tags = set()

            def drain(budget_ns):
                while budget_ns > 0 and filler:
                    cost, fn, tag = filler.popleft()
                    fn()
                    if tag is not None:
                        emitted_tags.add(tag)
                    budget_ns -= cost

            def drain_all():
                drain(float("inf"))

            def drain_until(tag):
                while tag not in emitted_tags and filler:
                    cost, fn, t = filler.popleft()
                    fn()
                    if t is not None:
                        emitted_tags.add(t)

            # ---------------- building blocks ----------------
            def enqueue_qkv(tt):
                """QKV for one 512-token tile: 6 chains (q,k,v x 2 pairs),
                each 8 accumulating matmuls of 512 cols."""
                xt = xt_tiles[tt]
                for pr in range(2):
                    for ob in range(3):  # q, k, v
                        ps_box = []

                        def make_item(pr, ob, h0, ps_box=None, xt=xt):
                            def emit():
                                if h0 == 0:
                                    ps_box.append(qkv_ps.tile(
                                        [128, 512], f32, tag="qkv",
                                        name="qkvp"))
                                ps = ps_box[0]
                                for hb in (h0, h0 + 1):
                                    if ob < 2:
                                        w = wqk_sb[:, 2048 * pr + 256 * hb
                                                   + 128 * ob:
                                                   2048 * pr + 256 * hb
                                                   + 128 * ob + 128]
                                    else:
                                        w = wv_sb[:, 1024 * pr + 128 * hb:
                                                  1024 * pr + 128 * hb + 128]
                                    nc.tensor.matmul(
                                        ps[:], w,
                                        xt[:, 512 * hb:512 * hb + 512],
                                        start=(hb == 0), stop=(hb == 7))
                                if h0 == 6:
                                    dst = (qT_sb, kT_sb, vT_sb)[ob][pr]
                                    if ob < 2:
                                        bias = qkb_sb[:, 2 * pr + ob:
                                                      2 * pr + ob + 1]
                                    else:
                                        bias = vb_sb[:, pr:pr + 1]
                                    nc.vector.tensor_scalar_add(
                                        dst[:, 512 * tt:512 * tt + 512],
                                        ps[:], bias)
                            return emit

                        for h0 in (0, 2, 4, 6):
                            filler.append(
                                (430, make_item(pr, ob, h0, ps_box),
                                 ("qkv", tt) if (pr == 1 and ob == 2
                                                 and h0 == 6) else None))

            def v_transpose(pr, tbs):
                """vT [d, t] -> v slots [t, d]: PE transpose + one strided
                DVE copy per 128-token block."""
                for tb in tbs:
                    slot = 16 * pr + tb
                    tcols = slice(128 * tb, 128 * tb + 128)
                    ps = qkv_ps.tile([128, 128], bf16, tag="qkv", name="vt")
                    nc.tensor.transpose(ps[:], vT_sb[pr][:, tcols], ident_sb[:])
                    dst = v_sb[:, 256 * slot:256 * slot + 256].rearrange(
                        "p (h x) -> p h x", h=2, x=128)[:, :, 0:64]
                    src = ps[:].rearrange("p (h x) -> p h x", h=2, x=64)
                    nc.vector.tensor_copy(dst, src)

            def enqueue_wo(att0, att1, qt):
                """Partial Wo for one q-quarter: contraction over BOTH pairs'
                att (256 dims): 2 accumulating matmuls per psum half."""
                for tb in range(4):
                    def make_item(tb, att0=att0, att1=att1, qt=qt):
                        def emit():
                            ps0 = qkv_ps.tile([128, 512], f32, tag="qkv",
                                              name="yps0")
                            nc.tensor.matmul(
                                ps0[:], att0[:, 128 * tb:128 * tb + 128],
                                wob_sb[:, 0:512], start=True, stop=False)
                            nc.tensor.matmul(
                                ps0[:], att1[:, 128 * tb:128 * tb + 128],
                                wob_sb[:, 1024:1536], start=False, stop=True)
                            ps1 = qkv_ps.tile([128, 512], f32, tag="qkv",
                                              name="yps1")
                            nc.tensor.matmul(
                                ps1[:], att0[:, 128 * tb:128 * tb + 128],
                                wob_sb[:, 512:1024], start=True, stop=False)
                            nc.tensor.matmul(
                                ps1[:], att1[:, 128 * tb:128 * tb + 128],
                                wob_sb[:, 1536:2048], start=False, stop=True)
                            ysb = y_pool.tile([128, 1024], bf16, tag="ysb",
                                              name="ysb")
                            nc.scalar.copy(ysb[:, 0:512], ps0[:])
                            nc.vector.tensor_copy(ysb[:, 512:1024], ps1[:])
                            row0 = 512 * qt + 128 * tb
                            eng = nc.gpsimd if tb % 2 == 0 else nc.sync
                            eng.dma_start(y_out[row0:row0 + 128, :], ysb[:])
                        return emit
                    filler.append((900, make_item(tb), None))

            def attention_slice(pr, qt):
                """Causal attention for head-pair pr, q rows [512qt, 512qt+512).
                Returns the normalized att tile [128 dims, 512 q]."""
                nkb = 4 * (qt + 1)
                avA = av_ps.tile([65, 512], f32, tag="avA", name="avA")
                avB = av_ps.tile([65, 512], f32, tag="avB", name="avB")
                qlo = 512 * qt
                pend = None  # AV runs one kb behind S^T/exp
                for kb in range(nkb):
                    klo = 128 * kb
                    roff = kb - 4 * qt
                    lo = 128 * roff if roff > 0 else 0
                    sAB = s_ps.tile([128, 1024], f32, tag="s", name="sAB")
                    nc.tensor.matmul(
                        sAB[:, lo:512], kT_sb[pr][0:64, klo:klo + 128],
                        qT_sb[pr][0:64, qlo + lo:qlo + 512])
                    nc.tensor.matmul(
                        sAB[:, 512:1024 - lo], kT_sb[pr][64:128, klo:klo + 128],
                        qT_sb[pr][64:128, qlo + lo:qlo + 512])
                    pAB = p_pool.tile([128, 1024], bf16, tag="p", name="pAB")
                    nc.scalar.activation(pAB[:, lo:1024 - lo],
                                         sAB[:, lo:1024 - lo],
                                         AF.Exp, scale=1.0 / math.sqrt(HD))
                    if roff >= 0:
                        c0 = 128 * roff
                        nc.gpsimd.tensor_mul(pAB[:, c0:c0 + 128],
                                             pAB[:, c0:c0 + 128], masks_sb[:])
                        nc.gpsimd.tensor_mul(pAB[:, 512:640],
                                             pAB[:, 512:640], masks_sb[:])
                    if pend is not None:
                        pkb, ppAB, plo = pend
                        pslot = 16 * pr + pkb
                        nc.tensor.matmul(
                            avA[:, plo:512],
                            v_sb[:, 256 * pslot:256 * pslot + 65],
                            ppAB[:, plo:512], start=(pkb == 0), stop=False)
                        nc.tensor.matmul(
                            avB[:, plo:512],
                            v_sb[:, 256 * pslot + 128:256 * pslot + 193],
                            ppAB[:, 512:1024 - plo], start=(pkb == 0),
                            stop=False)
                    pend = (kb, pAB, lo)
                    drain(600)
                pkb, ppAB, plo = pend
                pslot = 16 * pr + pkb
                nc.tensor.matmul(
                    avA[:, plo:512], v_sb[:, 256 * pslot:256 * pslot + 65],
                    ppAB[:, plo:512], start=(pkb == 0), stop=True)
                nc.tensor.matmul(
                    avB[:, plo:512], v_sb[:, 256 * pslot + 128:256 * pslot + 193],
                    ppAB[:, 512:1024 - plo], start=(pkb == 0), stop=True)

                # normalize: att_h = av_h[0:64] / sum_h  (sums in row 64).
                # rcp goes through qkv_ps (not s_ps) so the NEXT slice's S
                # matmuls don't wait on this slice's normalize chain.
                lrowA = rec_pool.tile([1, 512], f32r, tag="lrowA", name="lrowA")
                nc.scalar.copy(lrowA[:], avA[64:65, :])
                lrowB = rec_pool.tile([1, 512], f32r, tag="lrowB", name="lrowB")
                nc.scalar.copy(lrowB[:], avB[64:65, :])
                rcpA_ps = qkv_ps.tile([64, 512], f32, tag="qkv", name="rcpA_ps")
                nc.tensor.matmul(rcpA_ps[:], ones_sb[:], lrowA[:],
                                 start=True, stop=True)
                rcpB_ps = qkv_ps.tile([64, 512], f32, tag="qkv", name="rcpB_ps")
                nc.tensor.matmul(rcpB_ps[:], ones_sb[:], lrowB[:],
                                 start=True, stop=True)
                rcp_sb = rec_pool.tile([64, 1024], f32, tag="rcp", name="rcp_sb")
                nc.vector.reciprocal_approx_fast(rcp_sb[:, 0:512], rcpA_ps[:])
                nc.vector.reciprocal_approx_fast(rcp_sb[:, 512:1024], rcpB_ps[:])
                att_sb = att_pool.tile([128, 512], bf16, tag="att", name="att")
                nc.vector.tensor_mul(att_sb[0:64, :], avA[0:64, :],
                                     rcp_sb[:, 0:512])
                tmpB = rec_pool.tile([64, 512], bf16, tag="tmpB", name="tmpB")
                nc.vector.tensor_mul(tmpB[:], avB[0:64, :], rcp_sb[:, 512:1024])
                nc.vector.tensor_copy(att_sb[64:128, :], tmpB[:])
                return att_sb

            # ---------------- emission ----------------
            enqueue_qkv(0)
            drain_all()
            for qt in range(4):
                if qt < 3:
                    if qt + 1 not in xt_tiles:
                        prefetch_xt(qt + 1)
                    enqueue_qkv(qt + 1)
                drain_until(("qkv", qt))
                v_transpose(0, range(4 * qt, 4 * qt + 4))
                att0 = attention_slice(0, qt)
                v_transpose(1, range(4 * qt, 4 * qt + 4))
                att1 = attention_slice(1, qt)
                enqueue_wo(att0, att1, qt)
            drain_all()
        ctx_lp.__exit__(None, None, None)

    nc.compile()
    return nc


def host_prep(x, Wqkv_w, Wqkv_b, Wo_w, Wo_b):
    import ml_dtypes
    bf16 = ml_dtypes.bfloat16

    x = np.asarray(x, np.float32)
    Wqkv_w = np.asarray(Wqkv_w, np.float32)
    Wqkv_b = np.asarray(Wqkv_b, np.float32)
    Wo_w = np.asarray(Wo_w, np.float32)

    masks = np.ascontiguousarray(
        (np.arange(128)[:, None] <= np.arange(128)[None, :])
    ).astype(bf16)
    ones1x64 = np.ones((1, 64), dtype=np.float32)
    identm = np.ascontiguousarray(np.eye(128, dtype=np.float32)).astype(bf16)

    # xTb per batch: [512, 4096] with row tt*128+p, col hb*512+j
    xTbs = []
    for b in range(B):
        xf = x[b]                              # [2048, 1024]
        xTbs.append(np.ascontiguousarray(
            xf.reshape(4, 512, 8, 128).transpose(0, 3, 2, 1).reshape(512, 4096)
        ).astype(bf16))

    in_maps = []
    for c in range(NCORES):
        b, g = c // 4, c % 4
        heads = [4 * g + j for j in range(4)]   # two pairs: (h0,h1),(h2,h3)
        wqk_parts, qkb_cols, wv_parts, vb_cols, wob_parts = [], [], [], [], []
        for pr in range(2):
            hA, hB = heads[2 * pr], heads[2 * pr + 1]
            rows_qk = np.r_[64 * hA:64 * hA + 64, 64 * hB:64 * hB + 64,
                            1024 + 64 * hA:1024 + 64 * hA + 64,
                            1024 + 64 * hB:1024 + 64 * hB + 64]
            Wsub = Wqkv_w[rows_qk]              # [256, 1024]
            wqk_parts.append(
                Wsub.reshape(2, 128, 8, 128).transpose(3, 2, 0, 1)
                .reshape(128, 2048))
            qkb_cols.append(Wqkv_b[rows_qk].reshape(2, 128).T)  # [128, 2]
            rows_v = np.r_[2048 + 64 * hA:2048 + 64 * hA + 64,
                           2048 + 64 * hB:2048 + 64 * hB + 64]
            Vsub = Wqkv_w[rows_v]               # [128, 1024]
            wv_parts.append(
                Vsub.reshape(128, 8, 128).transpose(2, 1, 0).reshape(128, 1024))
            vb_cols.append(Wqkv_b[rows_v].reshape(128, 1))
            rows_o = np.r_[64 * hA:64 * hA + 64, 64 * hB:64 * hB + 64]
            wob_parts.append(Wo_w[:, rows_o].T)  # [128, 1024]
        wqkb = np.ascontiguousarray(
            np.concatenate(wqk_parts, axis=1)).astype(bf16)       # [128, 4096]
        qkbias = np.ascontiguousarray(np.concatenate(qkb_cols, axis=1))
        wvb = np.ascontiguousarray(
            np.concatenate(wv_parts, axis=1)).astype(bf16)        # [128, 2048]
        vbcol = np.ascontiguousarray(np.concatenate(vb_cols, axis=1))
        wob = np.ascontiguousarray(
            np.concatenate(wob_parts, axis=1)).astype(bf16)       # [128, 2048]
        in_maps.append(dict(
            xTb=xTbs[b], wqkb=wqkb, qkb=qkbias, wvb=wvb, vbcol=vbcol,
            masks128=masks, ones1x64=ones1x64, wob=wob, ident=identm))
    return in_maps


def _ensure_ntff_hook_module():
    """run_bass_kernel_spmd(trace=True) under axon imports
    antenv.axon_hooks; provide a ctypes-based fallback if absent."""
    import importlib
    import sys
    import types
    try:
        importlib.import_module("antenv.axon_hooks")
        return
    except ImportError:
        pass
    import contextlib
    import ctypes

    mod = types.ModuleType("antenv.axon_hooks")
    state = {"hook": None}

    def set_axon_ntff_profile_hook(h):
        state["hook"] = h

    def _make():
        try:
            lib = ctypes.CDLL("/opt/axon/libaxon_pjrt.so")
        except OSError:
            return None
        if not hasattr(lib, "axon_start_nrt_profile"):
            return None
        lib.axon_start_nrt_profile.argtypes = [
            ctypes.POINTER(ctypes.c_int64), ctypes.c_size_t]
        lib.axon_start_nrt_profile.restype = ctypes.c_int64
        lib.axon_stop_nrt_profile.argtypes = [ctypes.c_char_p]
        lib.axon_stop_nrt_profile.restype = ctypes.c_int64

        @contextlib.contextmanager
        def _hook(output_dir, device_ids):
            import jax
            jax.devices()
            if device_ids:
                ids = (ctypes.c_int64 * len(device_ids))(*device_ids)
                rc = lib.axon_start_nrt_profile(ids, len(device_ids))
            else:
                rc = lib.axon_start_nrt_profile(None, 0)
            if rc != 0:
                raise RuntimeError(f"axon_start_nrt_profile rc={rc}")
            try:
                yield
            finally:
                lib.axon_stop_nrt_profile(str(output_dir).encode())

        return _hook

    def get_axon_ntff_profile_hook():
        if state["hook"] is None:
            state["hook"] = _make()
        return state["hook"]

    mod.set_axon_ntff_profile_hook = set_axon_ntff_profile_hook
    mod.get_axon_ntff_profile_hook = get_axon_ntff_profile_hook
    try:
        import antenv
        sys.modules["antenv.axon_hooks"] = mod
        antenv.axon_hooks = mod
    except ImportError:
        pass


def kernel(x, Wqkv_w, Wqkv_b, Wo_w, Wo_b):
    from concourse import bass_utils

    _ensure_ntff_hook_module()

    if "nc" not in _cache:
        _cache["nc"] = _build()
    nc = _cache["nc"]

    in_maps = host_prep(x, Wqkv_w, Wqkv_b, Wo_w, Wo_b)
    res = bass_utils.run_bass_kernel_spmd(nc, in_maps, core_ids=list(range(NCORES)))
    _cache["last_results"] = res

    out = np.zeros((B, C, H), np.float32)
    for c in range(NCORES):
        b = c // 4
        out[b] += res.results[c]["y"].astype(np.float32)
    out += np.asarray(Wo_b, np.float32)[None, None, :]
    return out
